# revision 5
# baseline (speedup 1.0000x reference)
"""Trainium2 Bass kernel for nn_CrossAttention_46462956208727.

Math note: K and V are projections of the single global token g broadcast
along N, so every row of K (and V) is identical per batch sample. The
attention scores are therefore constant along the key axis, softmax is
exactly uniform, and attended == V's (identical) row. The whole module
collapses to

    out[b, n, :] = (g[b, 0, :] @ Wv + bv) @ Wo + bo        (independent of n, x)

This is a structural identity of the module (holds for any input values),
so the kernel computes the two tiny matmuls per sample on-device and
broadcasts the resulting 512-vector over the 4096 output rows. The
kernel is output-DMA bound: 8 MiB of HBM writes per core (~23 us at
~360 GB/s); everything else is a few microseconds of latency.

Sharding: data-parallel over B across the 8 cores (B == 8, one point
cloud per core); weights replicated.

Toolchain note: built on bacc.Bacc (not bass.Bass) and finalized before
dispatch — Bacc's compile pipeline runs generate_event_semaphores(),
which legalizes multi-semaphore waits into EventSemaphore predecessors
(walrus codegen allows only one sync-wait on most instruction structs).
"""

import numpy as np

import concourse.bacc as bacc
import concourse.tile as tile
from concourse import mybir
from concourse.bass_utils import run_bass_kernel_spmd

B, N = 8, 4096
LOCAL, GLOBAL, HIDDEN = 512, 128, 256
N_CORES = 8
P = 128
F32 = mybir.dt.float32

KC = HIDDEN // P        # 2 column-chunks of v (contraction split for v @ Wo)
REP = 4                 # row replicas per partition in the staging tile
FREE = REP * LOCAL      # 2048 f32 = 8 KiB per partition
NI = N // (P * REP)     # broadcast factor of the single output DMA (8)

_CACHE: dict = {}
LAST_RESULTS = None  # introspection for test harness (exec time, profile)


def _build_bass() -> bacc.Bacc:
    nc = bacc.Bacc(
        "TRN2", target_bir_lowering=False, debug=False, num_devices=N_CORES
    )
    g = nc.declare_dram_parameter("g", [GLOBAL], F32, isOutput=False)
    Wv = nc.declare_dram_parameter("Wv", [GLOBAL, HIDDEN], F32, isOutput=False)
    bv = nc.declare_dram_parameter("bv", [HIDDEN], F32, isOutput=False)
    Wo = nc.declare_dram_parameter("Wo", [HIDDEN, LOCAL], F32, isOutput=False)
    bo = nc.declare_dram_parameter("bo", [LOCAL], F32, isOutput=False)
    out = nc.declare_dram_parameter("out", [N, LOCAL], F32, isOutput=True)

    with tile.TileContext(nc) as tc:
        with (
            tc.tile_pool(name="w", bufs=1) as wpool,
            tc.tile_pool(name="ps", bufs=1, space="PSUM") as psum,
            tc.tile_pool(name="st", bufs=1) as spool,
        ):
            # ---- DMA loads --------------------------------------------------
            gT = wpool.tile([P, 1], F32)  # g as a column across partitions
            nc.sync.dma_start(out=gT[:], in_=g.ap().rearrange("(k o) -> k o", o=1))
            Wv_s = wpool.tile([P, HIDDEN], F32)
            nc.sync.dma_start(out=Wv_s[:], in_=Wv.ap())
            bv_s = wpool.tile([1, HIDDEN], F32)
            nc.sync.dma_start(out=bv_s[:], in_=bv.ap().rearrange("(o c) -> o c", o=1))
            Wo_s = wpool.tile([P, KC * LOCAL], F32)  # chunk c = Wo[c*128:(c+1)*128, :]
            for c in range(KC):
                nc.sync.dma_start(
                    out=Wo_s[:, c * LOCAL : (c + 1) * LOCAL],
                    in_=Wo.ap()[c * P : (c + 1) * P, :],
                )
            bo_s = wpool.tile([1, LOCAL], F32)
            nc.sync.dma_start(out=bo_s[:], in_=bo.ap().rearrange("(o c) -> o c", o=1))
            ones_s = wpool.tile([1, P], F32)
            nc.vector.memset(ones_s[:], 1.0)
            one_s = wpool.tile([1, 1], F32)
            nc.vector.memset(one_s[:], 1.0)

            # ---- vT = (g @ Wv + bv)^T as (128, KC) --------------------------
            vT_p = psum.tile([P, KC], F32)
            for c in range(KC):
                nc.tensor.matmul(
                    vT_p[:, c : c + 1],
                    lhsT=Wv_s[:, c * P : (c + 1) * P],
                    rhs=gT[:],
                    start=True,
                    stop=False,
                )
                # += bv chunk via K=1 outer product with a scalar 1
                nc.tensor.matmul(
                    vT_p[:, c : c + 1],
                    lhsT=bv_s[:, c * P : (c + 1) * P],
                    rhs=one_s[:],
                    start=False,
                    stop=True,
                )
            vT_s = spool.tile([P, KC], F32)
            nc.vector.tensor_copy(vT_s[:], vT_p[:])

            # ---- row = v @ Wo + bo as (1, LOCAL) ----------------------------
            row_p = psum.tile([1, LOCAL], F32)
            for c in range(KC):
                nc.tensor.matmul(
                    row_p[:],
                    lhsT=vT_s[:, c : c + 1],
                    rhs=Wo_s[:, c * LOCAL : (c + 1) * LOCAL],
                    start=(c == 0),
                    stop=(c == KC - 1),
                )
            row_s = spool.tile([1, LOCAL], F32)
            nc.vector.tensor_add(row_s[:], row_p[:], bo_s[:])

            # ---- broadcast row to all partitions: ones^T (x) row ------------
            bc_p = psum.tile([P, LOCAL], F32)
            nc.tensor.matmul(bc_p[:], lhsT=ones_s[:], rhs=row_s[:], start=True, stop=True)

            # ---- stage (128, FREE): row replicated REP times per partition --
            stage = spool.tile([P, FREE], F32)
            nc.vector.tensor_copy(stage[:, 0:LOCAL], bc_p[:])
            nc.vector.tensor_copy(stage[:, LOCAL : 2 * LOCAL], stage[:, 0:LOCAL])
            nc.vector.tensor_copy(
                stage[:, 2 * LOCAL : 4 * LOCAL], stage[:, 0 : 2 * LOCAL]
            )

            # ---- single 8 MiB store: DMA re-reads the stage tile NI times ---
            # partition p owns DRAM rows [32p, 32p+32) = 64 KiB contiguous
            out_v = out.ap().rearrange("(p i x) c -> p i (x c)", p=P, i=NI, x=REP)
            in_v = (
                stage[:]
                .rearrange("p (o f) -> p o f", o=1)
                .broadcast_to([P, NI, FREE])
            )
            nc.sync.dma_start(out=out_v, in_=in_v)
    nc.finalize()
    return nc


def kernel(**inputs) -> np.ndarray:
    global LAST_RESULTS
    g = np.ascontiguousarray(np.asarray(inputs["g"], dtype=np.float32))
    Wv = np.ascontiguousarray(np.asarray(inputs["Wv"], dtype=np.float32))
    bv = np.ascontiguousarray(np.asarray(inputs["bv"], dtype=np.float32))
    Wo = np.ascontiguousarray(np.asarray(inputs["Wo"], dtype=np.float32))
    bo = np.ascontiguousarray(np.asarray(inputs["bo"], dtype=np.float32))
    assert g.shape == (B, 1, GLOBAL), g.shape

    if "nc" not in _CACHE:
        _CACHE["nc"] = _build_bass()
    nc = _CACHE["nc"]

    in_maps = [
        {
            "g": g[c, 0],  # (GLOBAL,)
            "Wv": Wv,      # (GLOBAL, HIDDEN)
            "bv": bv,      # (HIDDEN,)
            "Wo": Wo,      # (HIDDEN, LOCAL)
            "bo": bo,      # (LOCAL,)
        }
        for c in range(N_CORES)
    ]
    res = run_bass_kernel_spmd(nc, in_maps, list(range(N_CORES)))
    LAST_RESULTS = res
    out = np.stack([res.results[c]["out"] for c in range(N_CORES)], axis=0)
    return np.ascontiguousarray(out, dtype=np.float32)
